# revision 1
# baseline (speedup 1.0000x reference)
"""Trainium2 Bass/Tile kernel: MoE-routed per-sample dynamic 3x3 conv (stride 2).

Reference computation:
    pooled  = mean(x, HW)                                        (B, Cin)
    rw      = sigmoid(pooled @ routing_w.T + routing_b)          (B, E)
    kernels = einsum('be,eoihw->boihw', rw, expert_weight)       (B,Cout,Cin,3,3)
    y[b]    = conv2d(x[b], kernels[b], stride 2, pad 1)          (B,Cout,56,56)

Sharding: data-parallel over batch across 8 NeuronCores (4 samples each);
routing/expert weights replicated (host pre-transposes them into the conv
lhsT layout [ci, e, tap, co]).  No collectives.

Per-core plan (software-pipelined across the 4 samples):
  - x[b] lives in SBUF as two half-sample slabs [128ci, 59, 112]: two zero
    dummy rows + fully contiguous rows (multi-KB DMA descriptors at full HBM
    rate).  Per-chunk DVE reduces compute the global-avg-pool as chunks land.
  - routing: pooled-col x routing_w^T matmul -> [1,E] logits; sigmoid on ACT;
    a K=1 ones-matmul broadcasts the 4 gate scalars to all 128 partitions.
  - combined per-sample conv weights W_b = sum_e rw[b,e] * E_r[e] on DVE, in
    (tap-row, co_tile) chunks ordered so the conv can start on the first one.
  - conv: out[co, oh, ow] accumulated in PSUM over the 9 taps; each matmul is
    lhsT=[ci,co_tile] (float32r, 1 row/cycle at N=392), rhs = strided slab
    view [ci, 7 rows, 56 cols].  The top pad row (ih=-1) reads the zero dummy
    row; the left pad (iw=-1) reads the previous row's column 111, and a
    small correction matmul computes exactly that garbage term so it can be
    subtracted from output column 0 during eviction.
"""

import numpy as np

try:
    import concourse.bass as bass
except ImportError:  # toolchain not on sys.path in a fresh interpreter
    import sys

    for _p in ("/opt/trn_rl_repo", "/root/.axon_site/_ro/trn_rl_repo"):
        if _p not in sys.path:
            sys.path.insert(0, _p)
    import concourse.bass as bass

import concourse.mybir as mybir
from concourse.bacc import Bacc
from concourse.bass_utils import run_bass_kernel_spmd
from concourse.masks import make_identity
from concourse.tile import TileContext

FP32 = mybir.dt.float32
F32R = mybir.dt.float32r

N_CORES = 8
B_FULL = 32
B_SH = B_FULL // N_CORES  # 4 samples per core
CIN = 128
H = W = 112
COUT = 256
E = 4
KH = KW = 3
OH = OW = 56
HWSZ = H * W  # 12544
R = 7  # output rows per PSUM block
NBLK = 8  # blocks per (sample, co_tile)
NN = R * OW  # 392 moving dim per matmul
S_ROWS = 59  # 2 zero dummy rows + up to 57 data rows
S_COLS = 112  # fully contiguous rows (multi-KB DMA descriptors)
DROW = 2  # x data starts at this slab row

# Tap order matches the weight-combine chunk order (dy=1 first).
TAPS = [(1, 1), (1, 0), (1, 2), (0, 1), (0, 0), (0, 2), (2, 1), (2, 0), (2, 2)]

_NC_CACHE = {}


def build_nc(rep=1):
    """Build the per-core module.  rep > 1 repeats the whole pipeline (same
    inputs/outputs) — used only for benchmarking slope measurements."""
    if rep in _NC_CACHE:
        return _NC_CACHE[rep]

    # Bacc (not raw Bass): its finalize() runs the legality passes this walrus
    # build needs — move_matmul_waits_to_ldweights + generate_event_semaphores
    # (max 1 sync wait per instruction) + register allocation.
    nc = Bacc(trn_type="TRN2")
    x = nc.dram_tensor("x", [B_SH, CIN, H, W], FP32, kind="ExternalInput")
    # weights arrive pre-transposed from the host (see make_in_maps):
    #   ew_t: [ci, e, tap, co]  (conv lhsT layout)   rw_t: [ci, e]
    rwt_h = nc.dram_tensor("routing_wt", [CIN, E], FP32, kind="ExternalInput")
    rb_h = nc.dram_tensor("routing_b", [E], FP32, kind="ExternalInput")
    ewt_h = nc.dram_tensor(
        "expert_weight_t", [CIN, E, KH * KW, COUT], FP32, kind="ExternalInput"
    )
    y = nc.dram_tensor("y", [B_SH, COUT, OH, OW], FP32, kind="ExternalOutput")

    with TileContext(nc) as tc:
        with (
            tc.tile_pool(name="const", bufs=1) as const,
            tc.tile_pool(name="slabs", bufs=4) as slabs,
            tc.tile_pool(name="wpool", bufs=2) as wpool,
            tc.tile_pool(name="stage", bufs=3) as stage,
            tc.tile_pool(name="small", bufs=2) as small,
            tc.tile_pool(name="ps_conv", bufs=2, space="PSUM") as ps_conv,
            tc.tile_pool(name="ps_rt", bufs=2, space="PSUM") as ps_rt,
        ):
            # ---------------- one-time prep ----------------
            ones_row = const.tile([1, 128], FP32)
            nc.vector.memset(ones_row, 1.0)
            bias_row = const.tile([1, E], FP32)
            nc.sync.dma_start(out=bias_row, in_=rb_h[:].unsqueeze(0))
            rwT = const.tile([128, E], FP32)
            nc.sync.dma_start(out=rwT, in_=rwt_h[:, :])

            # expert weights in lhsT layout [ci, e, tap, co], loaded directly
            e_r = const.tile([128, E, KH * KW, COUT], FP32)

            def emit_expert_prep():
                # tap-group major, d=1 first: the conv consumes d=1 taps first
                for d in (1, 0, 2):
                    nc.sync.dma_start(
                        out=e_r[:, :, 3 * d : 3 * d + 3, :],
                        in_=ewt_h[:, :, 3 * d : 3 * d + 3, :],
                    )

            # ---------------- per-sample pipeline ----------------
            # Emission is software-pipelined: sample b+1's loads + routing +
            # weight combine are emitted (= get scheduler priority) before
            # sample b's conv, so they execute under the previous conv.
            state = {}
            gstate = {}

            def emit_loads(b):
                slab0 = slabs.tile(
                    [128, S_ROWS, S_COLS], FP32, tag="slab", name=f"slab0_{b}"
                )
                slab1 = slabs.tile(
                    [128, S_ROWS, S_COLS], FP32, tag="slab", name=f"slab1_{b}"
                )
                # Fully contiguous loads (descriptors of 14 rows = 6.3 KB):
                #   slab0 rows 2..57 <- x rows 0..55; slab1 rows 2..58 <- 55..111
                # Rows 0..1 are memset to zero: row 1 is the conv's top pad
                # (ih = -1) and row DROW-1's column 111 doubles as the left pad
                # (iw = -1) for the first data row.  For the remaining rows the
                # dx=0 taps read the previous row's column 111 (garbage); a
                # per-co-tile correction matmul subtracts exactly that term
                # from output column 0 later.
                nc.gpsimd.memset(slab0[:, 0:DROW, :], 0.0)
                nc.gpsimd.memset(slab1[:, 0:DROW, :], 0.0)
                pooled = small.tile(
                    [128, 9], FP32, tag="pooled", name=f"pooled_{b}"
                )
                for c0 in range(4):
                    r0 = 14 * c0
                    nc.sync.dma_start(
                        out=slab0[:, DROW + r0 : DROW + r0 + 14, :].bitcast(F32R),
                        in_=x[b % B_SH, :, r0 : r0 + 14, :].bitcast(F32R),
                    )
                    nc.vector.tensor_reduce(
                        out=pooled[:, c0 : c0 + 1],
                        in_=slab0[:, DROW + r0 : DROW + r0 + 14, :],
                        axis=mybir.AxisListType.XY,
                        op=mybir.AluOpType.add,
                    )
                for c0 in range(4):
                    r0 = 14 * c0
                    nr = 14 if c0 < 3 else 15
                    nc.sync.dma_start(
                        out=slab1[:, DROW + r0 : DROW + r0 + nr, :].bitcast(F32R),
                        in_=x[b % B_SH, :, 55 + r0 : 55 + r0 + nr, :].bitcast(F32R),
                    )
                    # slab1 row DROW duplicates x row 55 -> skip it in the pool
                    rr0 = DROW + 1 if c0 == 0 else DROW + r0
                    nc.vector.tensor_reduce(
                        out=pooled[:, 4 + c0 : 5 + c0],
                        in_=slab1[:, rr0 : DROW + r0 + nr, :],
                        axis=mybir.AxisListType.XY,
                        op=mybir.AluOpType.add,
                    )
                gstate[b] = pooled
                state[b] = (slab0, slab1, None)

            def emit_gates(b):
                pooled = gstate.pop(b)
                slab0, slab1, _ = state[b]
                nc.vector.tensor_reduce(
                    out=pooled[:, 8:9],
                    in_=pooled[:, 0:8],
                    axis=mybir.AxisListType.X,
                    op=mybir.AluOpType.add,
                )

                # routing gates
                lg_ps = ps_rt.tile([1, E], FP32, tag="pr", name=f"lg_{b}")
                nc.tensor.matmul(lg_ps, pooled[:, 8:9], rwT, start=True, stop=True)
                lg_sb = small.tile([1, E], FP32, tag="lg", name=f"lgs_{b}")
                nc.vector.scalar_tensor_tensor(
                    out=lg_sb,
                    in0=lg_ps,
                    scalar=1.0 / HWSZ,
                    in1=bias_row,
                    op0=mybir.AluOpType.mult,
                    op1=mybir.AluOpType.add,
                )
                sig = small.tile([1, E], FP32, tag="sig", name=f"sig_{b}")
                nc.scalar.activation(
                    out=sig, in_=lg_sb, func=mybir.ActivationFunctionType.Sigmoid
                )
                bc_ps = ps_rt.tile([128, E], FP32, tag="pr", name=f"bc_{b}")
                nc.tensor.matmul(bc_ps, ones_row, sig, start=True, stop=True)
                rw_sb = small.tile([128, E], FP32, tag="rws", name=f"rws_{b}")
                nc.scalar.copy(out=rw_sb, in_=bc_ps)

                # combined per-sample conv weights, in (tap-row, co_tile)
                # chunks ordered to match conv consumption, so the conv can
                # start as soon as the first chunk lands.
                wb = wpool.tile([128, KH * KW, COUT], FP32, tag="wb", name=f"wb_{b}")
                for ct in range(2):
                    for d in (1, 0, 2):
                        dstf = wb[:, 3 * d : 3 * d + 3, ct * 128 : (ct + 1) * 128]
                        srcs = [
                            e_r[:, e, 3 * d : 3 * d + 3, ct * 128 : (ct + 1) * 128]
                            for e in range(E)
                        ]
                        nc.vector.tensor_scalar_mul(
                            out=dstf.bitcast(F32R),
                            in0=srcs[0],
                            scalar1=rw_sb[:, 0:1],
                        )
                        for e in range(1, E):
                            nc.vector.scalar_tensor_tensor(
                                out=dstf.bitcast(F32R),
                                in0=srcs[e],
                                scalar=rw_sb[:, e : e + 1],
                                in1=dstf,
                                op0=mybir.AluOpType.mult,
                                op1=mybir.AluOpType.add,
                            )
                state[b] = (slab0, slab1, wb)

            cstate = {}

            def emit_corrections(b):
                slab0, slab1, wb = state[b]
                allc = []
                for ct in range(2):
                    # dx=0 garbage corrections: C[co, oh] = sum_dy W(dy,0)^T .
                    # slab[prev-row col 111], one 28-wide run per half-sample.
                    c_sb = []
                    for run, sl in enumerate((slab0, slab1)):
                        c_ps = ps_rt.tile(
                            [128, 28], FP32, tag="pr", name=f"cps_{b}_{ct}_{run}"
                        )
                        fv = sl[:]
                        for di, dy in enumerate((0, 1, 2)):
                            lhsT = wb[
                                :, dy * 3, ct * 128 : (ct + 1) * 128
                            ].bitcast(F32R)
                            # garbage row for output row oh: 2*oh + dy (+1 on
                            # slab1) + (DROW - 1), column 111
                            roff = dy + (1 if run else 0)
                            rhs = bass.AP(
                                tensor=fv.tensor,
                                offset=fv.offset + roff * S_COLS + 111,
                                ap=[[fv.ap[0][0], 128], [2 * S_COLS, 28]],
                            ).bitcast(F32R)
                            nc.tensor.matmul(
                                c_ps[:, 0:28],
                                lhsT,
                                rhs,
                                start=(di == 0),
                                stop=(di == 2),
                            )
                        cs = small.tile(
                            [128, 28],
                            FP32,
                            tag="csb",
                            bufs=4,
                            name=f"cs_{b}_{ct}_{run}",
                        )
                        nc.scalar.copy(out=cs, in_=c_ps)
                        c_sb.append(cs)
                    allc.append(c_sb)
                cstate[b] = allc

            def emit_conv(b):
                slab0, slab1, wb = state.pop(b)
                allc = cstate.pop(b)
                for ct in range(2):
                    c_sb = allc[ct]
                    for g0, ng in ((0, 3), (3, 3), (6, 2)):
                        ps = ps_conv.tile(
                            [128, 3, 512], FP32, tag="pc", name=f"ps_{b}_{ct}_{g0}"
                        )
                        for ti, (dy, dx) in enumerate(TAPS):
                            lhsT = wb[
                                :, dy * 3 + dx, ct * 128 : (ct + 1) * 128
                            ].bitcast(F32R)
                            for j in range(ng):
                                i = g0 + j
                                sl = slab0 if i < 4 else slab1
                                fv = sl[:]
                                # slab row for output row r of block i:
                                #   14*(i%4) + 2r + dy + DROW - 1 (slab0)
                                #   14*(i%4) + 2r + dy + DROW     (slab1)
                                sr = (
                                    14 * (i % 4)
                                    + dy
                                    + (DROW - 1 if i < 4 else DROW)
                                )
                                rhs = bass.AP(
                                    tensor=fv.tensor,
                                    offset=fv.offset + sr * S_COLS + dx - 1,
                                    ap=[
                                        [fv.ap[0][0], 128],
                                        [2 * S_COLS, R],
                                        [2, OW],
                                    ],
                                ).bitcast(F32R)
                                nc.tensor.matmul(
                                    ps[:, j, 0:NN],
                                    lhsT,
                                    rhs,
                                    start=(ti == 0),
                                    stop=(ti == KH * KW - 1),
                                )
                        # evict, subtract the dx=0 garbage from column 0 of
                        # each output row, then write out
                        st = stage.tile(
                            [128, 3, NN], FP32, tag="st", name=f"st_{b}_{ct}_{g0}"
                        )
                        nc.scalar.copy(out=st[:, 0:ng, :], in_=ps[:, 0:ng, 0:NN])
                        col0 = st[:, 0:ng, 0 : 6 * OW + 1 : OW]
                        if g0 == 0:
                            fix = [(col0, c_sb[0][:, 0:21], 3)]
                        elif g0 == 3:
                            fix = [
                                (st[:, 0:1, 0 : 6 * OW + 1 : OW], c_sb[0][:, 21:28], 1),
                                (st[:, 1:3, 0 : 6 * OW + 1 : OW], c_sb[1][:, 0:14], 2),
                            ]
                        else:
                            fix = [(st[:, 0:2, 0 : 6 * OW + 1 : OW], c_sb[1][:, 14:28], 2)]
                        for dst_v, c_v, nb_ in fix:
                            nc.vector.tensor_sub(
                                out=dst_v,
                                in0=dst_v,
                                in1=c_v.rearrange("p (a c) -> p a c", c=R),
                            )
                        yv = y[b % B_SH, ct * 128 : (ct + 1) * 128, :, :].rearrange(
                            "p a c -> p (a c)"
                        )
                        nc.sync.dma_start(
                            out=yv[:, g0 * NN : (g0 + ng) * NN],
                            in_=st[:, 0:ng, :].rearrange("p a c -> p (a c)"),
                        )

            nb = B_SH * rep
            emit_loads(0)
            emit_expert_prep()
            emit_gates(0)
            for b in range(nb):
                emit_corrections(b)
                if b + 1 < nb:
                    emit_loads(b + 1)
                    emit_gates(b + 1)
                emit_conv(b)

    nc.finalize()
    _NC_CACHE[rep] = nc
    return nc


def make_in_maps(x, routing_w, routing_b, expert_weight):
    x = np.ascontiguousarray(np.asarray(x, dtype=np.float32))
    routing_w = np.asarray(routing_w, dtype=np.float32)
    routing_b = np.ascontiguousarray(np.asarray(routing_b, dtype=np.float32))
    expert_weight = np.asarray(expert_weight, dtype=np.float32)
    # host-side weight re-layout (replicated across cores):
    #   expert_weight [e, co, ci, kh, kw] -> [ci, e, kh*kw, co]
    ew_t = np.ascontiguousarray(
        expert_weight.transpose(2, 0, 3, 4, 1).reshape(CIN, E, KH * KW, COUT)
    )
    rw_t = np.ascontiguousarray(routing_w.T)
    return [
        {
            "x": np.ascontiguousarray(x[c * B_SH : (c + 1) * B_SH]),
            "routing_wt": rw_t,
            "routing_b": routing_b,
            "expert_weight_t": ew_t,
        }
        for c in range(N_CORES)
    ]


def kernel(x, routing_w, routing_b, expert_weight):
    nc = build_nc()
    in_maps = make_in_maps(x, routing_w, routing_b, expert_weight)
    res = run_bass_kernel_spmd(nc, in_maps, core_ids=list(range(N_CORES)))
    return np.concatenate([res.results[c]["y"] for c in range(N_CORES)], axis=0)



# revision 2
# speedup vs baseline: 1.1567x; 1.1567x over previous
"""Trainium2 Bass/Tile kernel: MoE-routed per-sample dynamic 3x3 conv (stride 2).

Reference computation:
    pooled  = mean(x, HW)                                        (B, Cin)
    rw      = sigmoid(pooled @ routing_w.T + routing_b)          (B, E)
    kernels = einsum('be,eoihw->boihw', rw, expert_weight)       (B,Cout,Cin,3,3)
    y[b]    = conv2d(x[b], kernels[b], stride 2, pad 1)          (B,Cout,56,56)

Sharding: data-parallel over batch across 8 NeuronCores (4 samples each);
routing/expert weights replicated.  No collectives.

bf16 edition: the problem is ridge-regime — at fp32 both the HBM roofline
(43 MB/core ~ 120us at 360 GB/s) and the PE roofline (225,792 matmul rows
~ 94us at 2.4 GHz) bind.  Host pre-casts x / expert weights to bf16 and the
kernel stores y as bf16 (upcast on host), cutting HBM traffic to ~22 MB/core
(~60us) so only the PE roofline remains.  bf16 matmuls run 1 row/cycle, same
as fp32r, so PE time is unchanged; end-to-end error is ~4e-3 << the 2e-2 gate.

Host additionally zero-pads x to [113, 114] (1 top pad row, 1 left pad col)
so the conv's padding reads hit real zeros: no dummy-row memsets and no
left-pad correction matmuls.  Each (sample, ci) plane is one fully
contiguous 25.8 KB DMA per partition.

Per-core plan (software-pipelined across the 4 samples):
  - x[b] slab [128ci, 113, 114] bf16 in SBUF; per-chunk DVE reduces compute
    the global-avg-pool as the 8 DMA chunks land.
  - routing: pooled-col x routing_w^T fp32 matmul -> [1,E] logits; sigmoid
    on ACT; a K=1 ones-matmul broadcasts the 4 gates to all 128 partitions.
  - combined per-sample conv weights W_b = sum_e rw[b,e] * E_r[e] on DVE
    (bf16), in (tap-row, co_tile) chunks ordered so the conv can start on
    the first one.
  - conv: out[co, oh, ow] accumulated in PSUM over the 9 taps; each matmul
    is lhsT=[ci,co_tile] bf16, rhs = strided slab view [ci, 7 rows, 56
    cols].  Padding reads hit host-written zeros.
"""

import numpy as np

try:
    import concourse.bass as bass
except ImportError:  # toolchain not on sys.path in a fresh interpreter
    import sys

    for _p in ("/opt/trn_rl_repo", "/root/.axon_site/_ro/trn_rl_repo"):
        if _p not in sys.path:
            sys.path.insert(0, _p)
    import concourse.bass as bass

import concourse.mybir as mybir
from concourse.bacc import Bacc
from concourse.bass_utils import run_bass_kernel_spmd
from concourse.tile import TileContext

FP32 = mybir.dt.float32
BF16 = mybir.dt.bfloat16
NP_BF16 = mybir.dt.np(BF16)

N_CORES = 8
B_FULL = 32
B_SH = B_FULL // N_CORES  # 4 samples per core
CIN = 128
H = W = 112
COUT = 256
E = 4
KH = KW = 3
OH = OW = 56
HWSZ = H * W  # 12544
R = 7  # output rows per PSUM block
NBLK = 8  # blocks per (sample, co_tile)
NN = R * OW  # 392 moving dim per matmul
SR = 113  # 1 zero pad row + 112 data rows
SC = 114  # 1 zero pad col + 112 data cols + 1 zero pad col

# Tap order matches the weight-combine chunk order (dy=1 first).
TAPS = [(1, 1), (1, 0), (1, 2), (0, 1), (0, 0), (0, 2), (2, 1), (2, 0), (2, 2)]

_NC_CACHE = {}


def build_nc(rep=1):
    """Build the per-core module.  rep > 1 repeats the whole pipeline (same
    inputs/outputs) — used only for benchmarking slope measurements."""
    if rep in _NC_CACHE:
        return _NC_CACHE[rep]

    nc = Bacc(trn_type="TRN2")
    # x arrives host-padded+bf16: [b, ci, 113, 114], row 0 / col 0 / col 113
    # are zeros (the conv's pad ring).
    x = nc.dram_tensor("x", [B_SH, CIN, SR, SC], BF16, kind="ExternalInput")
    rwt_h = nc.dram_tensor("routing_wt", [CIN, E], FP32, kind="ExternalInput")
    rb_h = nc.dram_tensor("routing_b", [E], FP32, kind="ExternalInput")
    # expert weights pre-transposed on host into conv lhsT layout [ci,e,tap,co]
    ewt_h = nc.dram_tensor(
        "expert_weight_t", [CIN, E, KH * KW, COUT], BF16, kind="ExternalInput"
    )
    y = nc.dram_tensor("y", [B_SH, COUT, OH, OW], BF16, kind="ExternalOutput")

    with TileContext(nc) as tc:
        with (
            tc.tile_pool(name="const", bufs=1) as const,
            tc.tile_pool(name="slabs", bufs=3) as slabs,
            tc.tile_pool(name="wpool", bufs=2) as wpool,
            tc.tile_pool(name="stage", bufs=3) as stage,
            tc.tile_pool(name="small", bufs=2) as small,
            tc.tile_pool(name="ps_conv", bufs=2, space="PSUM") as ps_conv,
            tc.tile_pool(name="ps_rt", bufs=2, space="PSUM") as ps_rt,
        ):
            # ---------------- one-time prep ----------------
            ones_row = const.tile([1, 128], FP32)
            nc.vector.memset(ones_row, 1.0)
            bias_row = const.tile([1, E], FP32)
            nc.sync.dma_start(out=bias_row, in_=rb_h[:].unsqueeze(0))
            rwT = const.tile([128, E], FP32)
            nc.sync.dma_start(out=rwT, in_=rwt_h[:, :])

            e_r = const.tile([128, E, KH * KW, COUT], BF16)

            def emit_expert_prep():
                # tap-group major, d=1 first: the conv consumes d=1 taps first
                for d in (1, 0, 2):
                    nc.sync.dma_start(
                        out=e_r[:, :, 3 * d : 3 * d + 3, :],
                        in_=ewt_h[:, :, 3 * d : 3 * d + 3, :],
                    )

            # ---------------- per-sample pipeline ----------------
            state = {}
            gstate = {}

            def emit_loads(b):
                slab = slabs.tile([128, SR, SC], BF16, tag="slab", name=f"slab_{b}")
                pooled = small.tile([128, 9], FP32, tag="pooled", name=f"pooled_{b}")
                for c0 in range(8):
                    r0 = 14 * c0
                    nr = 14 if c0 < 7 else 15
                    nc.sync.dma_start(
                        out=slab[:, r0 : r0 + nr, :],
                        in_=x[b % B_SH, :, r0 : r0 + nr, :],
                    )
                    # pad zeros contribute nothing to the sum
                    nc.vector.tensor_reduce(
                        out=pooled[:, c0 : c0 + 1],
                        in_=slab[:, r0 : r0 + nr, :],
                        axis=mybir.AxisListType.XY,
                        op=mybir.AluOpType.add,
                    )
                gstate[b] = pooled
                state[b] = (slab, None)

            def emit_gates_lg(b):
                # logits + sigmoid: ends on ACT so the PE stream only stalls
                # on the (already-loaded) pooled column
                pooled = gstate.pop(b)
                nc.vector.tensor_reduce(
                    out=pooled[:, 8:9],
                    in_=pooled[:, 0:8],
                    axis=mybir.AxisListType.X,
                    op=mybir.AluOpType.add,
                )
                lg_ps = ps_rt.tile([1, E], FP32, tag="pr", name=f"lg_{b}")
                nc.tensor.matmul(lg_ps, pooled[:, 8:9], rwT, start=True, stop=True)
                lg_sb = small.tile([1, E], FP32, tag="lg", name=f"lgs_{b}")
                nc.vector.scalar_tensor_tensor(
                    out=lg_sb,
                    in0=lg_ps,
                    scalar=1.0 / HWSZ,
                    in1=bias_row,
                    op0=mybir.AluOpType.mult,
                    op1=mybir.AluOpType.add,
                )
                sig = small.tile([1, E], FP32, tag="sig", name=f"sig_{b}")
                nc.scalar.activation(
                    out=sig, in_=lg_sb, func=mybir.ActivationFunctionType.Sigmoid
                )
                gstate[b] = sig

            def emit_gates_comb(b):
                # broadcast the 4 gates to all partitions, then combine the
                # per-sample conv weights on DVE; emitted after the first conv
                # group of the previous sample so the sigmoid chain hides
                # under PE work
                sig = gstate.pop(b)
                slab, _ = state[b]
                bc_ps = ps_rt.tile([128, E], FP32, tag="pr", name=f"bc_{b}")
                nc.tensor.matmul(bc_ps, ones_row, sig, start=True, stop=True)
                rw_sb = small.tile([128, E], FP32, tag="rws", name=f"rws_{b}")
                nc.scalar.copy(out=rw_sb, in_=bc_ps)

                wb = wpool.tile([128, KH * KW, COUT], BF16, tag="wb", name=f"wb_{b}")
                for ct in range(2):
                    for d in (1, 0, 2):
                        dstf = wb[:, 3 * d : 3 * d + 3, ct * 128 : (ct + 1) * 128]
                        srcs = [
                            e_r[:, e, 3 * d : 3 * d + 3, ct * 128 : (ct + 1) * 128]
                            for e in range(E)
                        ]
                        nc.vector.tensor_scalar_mul(
                            out=dstf,
                            in0=srcs[0],
                            scalar1=rw_sb[:, 0:1],
                        )
                        for e in range(1, E):
                            nc.vector.scalar_tensor_tensor(
                                out=dstf,
                                in0=srcs[e],
                                scalar=rw_sb[:, e : e + 1],
                                in1=dstf,
                                op0=mybir.AluOpType.mult,
                                op1=mybir.AluOpType.add,
                            )
                state[b] = (slab, wb)

            def emit_conv(b, groups, last=False):
                slab, wb = state[b] if not last else state.pop(b)
                for ct, g0, ng in groups:
                        ps = ps_conv.tile(
                            [128, 3, 512], FP32, tag="pc", name=f"ps_{b}_{ct}_{g0}"
                        )
                        for ti, (dy, dx) in enumerate(TAPS):
                            lhsT = wb[:, dy * 3 + dx, ct * 128 : (ct + 1) * 128]
                            for j in range(ng):
                                i = g0 + j
                                # block i = output rows 7i..7i+6; slab row for
                                # output row oh, tap dy is 2*oh + dy (pad row 0)
                                fv = slab[:]
                                rhs = bass.AP(
                                    tensor=fv.tensor,
                                    offset=fv.offset + (14 * i + dy) * SC + dx,
                                    ap=[
                                        [fv.ap[0][0], 128],
                                        [2 * SC, R],
                                        [2, OW],
                                    ],
                                )
                                nc.tensor.matmul(
                                    ps[:, j, 0:NN],
                                    lhsT,
                                    rhs,
                                    start=(ti == 0),
                                    stop=(ti == KH * KW - 1),
                                )
                        st = stage.tile(
                            [128, 3, NN], BF16, tag="st", name=f"st_{b}_{ct}_{g0}"
                        )
                        nc.scalar.copy(out=st[:, 0:ng, :], in_=ps[:, 0:ng, 0:NN])
                        yv = y[b % B_SH, ct * 128 : (ct + 1) * 128, :, :].rearrange(
                            "p a c -> p (a c)"
                        )
                        nc.sync.dma_start(
                            out=yv[:, g0 * NN : (g0 + ng) * NN],
                            in_=st[:, 0:ng, :].rearrange("p a c -> p (a c)"),
                        )

            GROUPS = [
                (0, 0, 3), (0, 3, 3), (0, 6, 2),
                (1, 0, 3), (1, 3, 3), (1, 6, 2),
            ]
            nb = B_SH * rep
            emit_loads(0)
            emit_expert_prep()
            emit_gates_lg(0)
            emit_gates_comb(0)
            for b in range(nb):
                if b + 1 < nb:
                    emit_loads(b + 1)
                    emit_gates_lg(b + 1)
                emit_conv(b, GROUPS[:1])
                if b + 1 < nb:
                    emit_gates_comb(b + 1)
                emit_conv(b, GROUPS[1:], last=True)

    nc.finalize()
    _NC_CACHE[rep] = nc
    return nc


def make_in_maps(x, routing_w, routing_b, expert_weight):
    x = np.asarray(x, dtype=np.float32)
    routing_w = np.asarray(routing_w, dtype=np.float32)
    routing_b = np.ascontiguousarray(np.asarray(routing_b, dtype=np.float32))
    expert_weight = np.asarray(expert_weight, dtype=np.float32)
    # host-side pad + bf16 cast of x: [B, ci, 113, 114], zeros in row 0 /
    # col 0 / col 113
    xp = np.zeros((B_FULL, CIN, SR, SC), dtype=NP_BF16)
    xp[:, :, 1:, 1 : 1 + W] = x.astype(NP_BF16)
    # expert_weight [e, co, ci, kh, kw] -> [ci, e, kh*kw, co], bf16
    ew_t = np.ascontiguousarray(
        expert_weight.transpose(2, 0, 3, 4, 1)
        .reshape(CIN, E, KH * KW, COUT)
        .astype(NP_BF16)
    )
    rw_t = np.ascontiguousarray(routing_w.T)
    return [
        {
            "x": np.ascontiguousarray(xp[c * B_SH : (c + 1) * B_SH]),
            "routing_wt": rw_t,
            "routing_b": routing_b,
            "expert_weight_t": ew_t,
        }
        for c in range(N_CORES)
    ]


def kernel(x, routing_w, routing_b, expert_weight):
    nc = build_nc()
    in_maps = make_in_maps(x, routing_w, routing_b, expert_weight)
    res = run_bass_kernel_spmd(nc, in_maps, core_ids=list(range(N_CORES)))
    return np.concatenate(
        [res.results[c]["y"].astype(np.float32) for c in range(N_CORES)], axis=0
    )


# revision 3
# speedup vs baseline: 1.2851x; 1.1110x over previous
"""Trainium2 Bass/Tile kernel: MoE-routed per-sample dynamic 3x3 conv (stride 2).

Reference computation:
    pooled  = mean(x, HW)                                        (B, Cin)
    rw      = sigmoid(pooled @ routing_w.T + routing_b)          (B, E)
    kernels = einsum('be,eoihw->boihw', rw, expert_weight)       (B,Cout,Cin,3,3)
    y[b]    = conv2d(x[b], kernels[b], stride 2, pad 1)          (B,Cout,56,56)

Sharding: data-parallel over batch across 8 NeuronCores (4 samples each);
routing/expert weights replicated.  No collectives.

bf16 edition: the problem is ridge-regime — at fp32 both the HBM roofline
(43 MB/core ~ 120us at 360 GB/s) and the PE roofline (225,792 matmul rows
~ 94us at 2.4 GHz) bind.  Host pre-casts x / expert weights to bf16 and the
kernel stores y as bf16 (upcast on host), cutting HBM traffic to ~22 MB/core
(~60us) so only the PE roofline remains.  bf16 matmuls run 1 row/cycle, same
as fp32r, so PE time is unchanged; end-to-end error is ~4e-3 << the 2e-2 gate.

Host additionally zero-pads x to [113, 114] (1 top pad row, 1 left pad col)
so the conv's padding reads hit real zeros: no dummy-row memsets and no
left-pad correction matmuls.  Each (sample, ci) plane is one fully
contiguous 25.8 KB DMA per partition.

Per-core plan (software-pipelined across the 4 samples):
  - x[b] slab [128ci, 113, 114] bf16 in SBUF (triple-buffered so sample b+1's
    loads run fully under sample b-1/b's conv); per-chunk DVE reduces compute
    the global-avg-pool as the 8 DMA chunks land.
  - routing: pooled-col x routing_w^T fp32 matmul -> [1,E] logits; sigmoid
    on ACT; a K=1 ones-matmul broadcasts the 4 gates to all 128 partitions.
    The logit half is emitted before sample b's conv and the broadcast +
    weight-combine after conv b's first PSUM group, so the DVE->ACT->PE
    sigmoid chain hides under conv matmuls instead of stalling the PE.
  - combined per-sample conv weights W_b = sum_e rw[b,e] * E_r[e] on DVE
    (bf16), in (tap-row, co_tile) chunks ordered so the conv can start on
    the first one.
  - conv: out[co, oh, ow] accumulated in PSUM over the 9 taps; each matmul
    is lhsT=[ci,co_tile] bf16, rhs = strided slab view [ci, 7 rows, 56
    cols].  Padding reads hit host-written zeros.
"""

import numpy as np

try:
    import concourse.bass as bass
except ImportError:  # toolchain not on sys.path in a fresh interpreter
    import sys

    for _p in ("/opt/trn_rl_repo", "/root/.axon_site/_ro/trn_rl_repo"):
        if _p not in sys.path:
            sys.path.insert(0, _p)
    import concourse.bass as bass

import concourse.mybir as mybir
from concourse.bacc import Bacc
from concourse.bass_utils import run_bass_kernel_spmd
from concourse.tile import TileContext

FP32 = mybir.dt.float32
BF16 = mybir.dt.bfloat16
NP_BF16 = mybir.dt.np(BF16)

N_CORES = 8
B_FULL = 32
B_SH = B_FULL // N_CORES  # 4 samples per core
CIN = 128
H = W = 112
COUT = 256
E = 4
KH = KW = 3
OH = OW = 56
HWSZ = H * W  # 12544
R = 7  # output rows per PSUM block
NBLK = 8  # blocks per (sample, co_tile)
NN = R * OW  # 392 moving dim per matmul
SR = 113  # 1 zero pad row + 112 data rows
SC = 114  # 1 zero pad col + 112 data cols + 1 zero pad col

# Tap order matches the weight-combine chunk order (dy=1 first).
TAPS = [(1, 1), (1, 0), (1, 2), (0, 1), (0, 0), (0, 2), (2, 1), (2, 0), (2, 2)]

_NC_CACHE = {}


def build_nc(rep=1):
    """Build the per-core module.  rep > 1 repeats the whole pipeline (same
    inputs/outputs) — used only for benchmarking slope measurements."""
    if rep in _NC_CACHE:
        return _NC_CACHE[rep]

    nc = Bacc(trn_type="TRN2")
    # x arrives host-padded+bf16: [b, ci, 113, 114], row 0 / col 0 / col 113
    # are zeros (the conv's pad ring).
    x = nc.dram_tensor("x", [B_SH, CIN, SR, SC], BF16, kind="ExternalInput")
    rwt_h = nc.dram_tensor("routing_wt", [CIN, E], FP32, kind="ExternalInput")
    rb_h = nc.dram_tensor("routing_b", [E], FP32, kind="ExternalInput")
    # expert weights pre-transposed on host into conv lhsT layout [ci,e,tap,co]
    ewt_h = nc.dram_tensor(
        "expert_weight_t", [CIN, E, KH * KW, COUT], BF16, kind="ExternalInput"
    )
    y = nc.dram_tensor("y", [B_SH, COUT, OH, OW], BF16, kind="ExternalOutput")

    with TileContext(nc) as tc:
        with (
            tc.tile_pool(name="const", bufs=1) as const,
            tc.tile_pool(name="slabs", bufs=3) as slabs,
            tc.tile_pool(name="wpool", bufs=2) as wpool,
            tc.tile_pool(name="stage", bufs=3) as stage,
            tc.tile_pool(name="small", bufs=2) as small,
            tc.tile_pool(name="ps_conv", bufs=2, space="PSUM") as ps_conv,
            tc.tile_pool(name="ps_rt", bufs=2, space="PSUM") as ps_rt,
        ):
            # ---------------- one-time prep ----------------
            ones_row = const.tile([1, 128], FP32)
            nc.vector.memset(ones_row, 1.0)
            bias_row = const.tile([1, E], FP32)
            nc.sync.dma_start(out=bias_row, in_=rb_h[:].unsqueeze(0))
            rwT = const.tile([128, E], FP32)
            nc.sync.dma_start(out=rwT, in_=rwt_h[:, :])

            e_r = const.tile([128, E, KH * KW, COUT], BF16)

            def emit_expert_prep():
                # tap-group major, d=1 first: the conv consumes d=1 taps first
                for d in (1, 0, 2):
                    nc.sync.dma_start(
                        out=e_r[:, :, 3 * d : 3 * d + 3, :],
                        in_=ewt_h[:, :, 3 * d : 3 * d + 3, :],
                    )

            # ---------------- per-sample pipeline ----------------
            state = {}
            gstate = {}

            def emit_loads(b):
                slab = slabs.tile([128, SR, SC], BF16, tag="slab", name=f"slab_{b}")
                pooled = small.tile([128, 9], FP32, tag="pooled", name=f"pooled_{b}")
                for c0 in range(8):
                    r0 = 14 * c0
                    nr = 14 if c0 < 7 else 15
                    nc.sync.dma_start(
                        out=slab[:, r0 : r0 + nr, :],
                        in_=x[b % B_SH, :, r0 : r0 + nr, :],
                    )
                    # pad zeros contribute nothing to the sum
                    nc.vector.tensor_reduce(
                        out=pooled[:, c0 : c0 + 1],
                        in_=slab[:, r0 : r0 + nr, :],
                        axis=mybir.AxisListType.XY,
                        op=mybir.AluOpType.add,
                    )
                gstate[b] = pooled
                state[b] = (slab, None)

            def emit_gates_lg(b):
                # logits + sigmoid: ends on ACT so the PE stream only stalls
                # on the (already-loaded) pooled column
                pooled = gstate.pop(b)
                nc.vector.tensor_reduce(
                    out=pooled[:, 8:9],
                    in_=pooled[:, 0:8],
                    axis=mybir.AxisListType.X,
                    op=mybir.AluOpType.add,
                )
                lg_ps = ps_rt.tile([1, E], FP32, tag="pr", name=f"lg_{b}")
                nc.tensor.matmul(lg_ps, pooled[:, 8:9], rwT, start=True, stop=True)
                lg_sb = small.tile([1, E], FP32, tag="lg", name=f"lgs_{b}")
                nc.vector.scalar_tensor_tensor(
                    out=lg_sb,
                    in0=lg_ps,
                    scalar=1.0 / HWSZ,
                    in1=bias_row,
                    op0=mybir.AluOpType.mult,
                    op1=mybir.AluOpType.add,
                )
                sig = small.tile([1, E], FP32, tag="sig", name=f"sig_{b}")
                nc.scalar.activation(
                    out=sig, in_=lg_sb, func=mybir.ActivationFunctionType.Sigmoid
                )
                gstate[b] = sig

            def emit_gates_comb(b):
                # broadcast the 4 gates to all partitions, then combine the
                # per-sample conv weights on DVE; emitted after the first conv
                # group of the previous sample so the sigmoid chain hides
                # under PE work
                sig = gstate.pop(b)
                slab, _ = state[b]
                bc_ps = ps_rt.tile([128, E], FP32, tag="pr", name=f"bc_{b}")
                nc.tensor.matmul(bc_ps, ones_row, sig, start=True, stop=True)
                rw_sb = small.tile([128, E], FP32, tag="rws", name=f"rws_{b}")
                nc.scalar.copy(out=rw_sb, in_=bc_ps)

                wb = wpool.tile([128, KH * KW, COUT], BF16, tag="wb", name=f"wb_{b}")
                for ct in range(2):
                    for d in (1, 0, 2):
                        dstf = wb[:, 3 * d : 3 * d + 3, ct * 128 : (ct + 1) * 128]
                        srcs = [
                            e_r[:, e, 3 * d : 3 * d + 3, ct * 128 : (ct + 1) * 128]
                            for e in range(E)
                        ]
                        nc.vector.tensor_scalar_mul(
                            out=dstf,
                            in0=srcs[0],
                            scalar1=rw_sb[:, 0:1],
                        )
                        for e in range(1, E):
                            nc.vector.scalar_tensor_tensor(
                                out=dstf,
                                in0=srcs[e],
                                scalar=rw_sb[:, e : e + 1],
                                in1=dstf,
                                op0=mybir.AluOpType.mult,
                                op1=mybir.AluOpType.add,
                            )
                state[b] = (slab, wb)

            def emit_conv(b, groups, last=False):
                slab, wb = state[b] if not last else state.pop(b)
                for ct, g0, ng in groups:
                        ps = ps_conv.tile(
                            [128, 3, 512], FP32, tag="pc", name=f"ps_{b}_{ct}_{g0}"
                        )
                        for ti, (dy, dx) in enumerate(TAPS):
                            lhsT = wb[:, dy * 3 + dx, ct * 128 : (ct + 1) * 128]
                            for j in range(ng):
                                i = g0 + j
                                # block i = output rows 7i..7i+6; slab row for
                                # output row oh, tap dy is 2*oh + dy (pad row 0)
                                fv = slab[:]
                                rhs = bass.AP(
                                    tensor=fv.tensor,
                                    offset=fv.offset + (14 * i + dy) * SC + dx,
                                    ap=[
                                        [fv.ap[0][0], 128],
                                        [2 * SC, R],
                                        [2, OW],
                                    ],
                                )
                                nc.tensor.matmul(
                                    ps[:, j, 0:NN],
                                    lhsT,
                                    rhs,
                                    start=(ti == 0),
                                    stop=(ti == KH * KW - 1),
                                )
                        st = stage.tile(
                            [128, 3, NN], BF16, tag="st", name=f"st_{b}_{ct}_{g0}"
                        )
                        nc.scalar.copy(out=st[:, 0:ng, :], in_=ps[:, 0:ng, 0:NN])
                        yv = y[b % B_SH, ct * 128 : (ct + 1) * 128, :, :].rearrange(
                            "p a c -> p (a c)"
                        )
                        nc.sync.dma_start(
                            out=yv[:, g0 * NN : (g0 + ng) * NN],
                            in_=st[:, 0:ng, :].rearrange("p a c -> p (a c)"),
                        )

            GROUPS = [
                (0, 0, 3), (0, 3, 3), (0, 6, 2),
                (1, 0, 3), (1, 3, 3), (1, 6, 2),
            ]
            nb = B_SH * rep
            emit_loads(0)
            emit_expert_prep()
            emit_gates_lg(0)
            emit_gates_comb(0)
            for b in range(nb):
                if b + 1 < nb:
                    emit_loads(b + 1)
                    emit_gates_lg(b + 1)
                emit_conv(b, GROUPS[:1])
                if b + 1 < nb:
                    emit_gates_comb(b + 1)
                emit_conv(b, GROUPS[1:], last=True)

    nc.finalize()
    _NC_CACHE[rep] = nc
    return nc


def make_in_maps(x, routing_w, routing_b, expert_weight):
    x = np.asarray(x, dtype=np.float32)
    routing_w = np.asarray(routing_w, dtype=np.float32)
    routing_b = np.ascontiguousarray(np.asarray(routing_b, dtype=np.float32))
    expert_weight = np.asarray(expert_weight, dtype=np.float32)
    # host-side pad + bf16 cast of x: [B, ci, 113, 114], zeros in row 0 /
    # col 0 / col 113
    xp = np.zeros((B_FULL, CIN, SR, SC), dtype=NP_BF16)
    xp[:, :, 1:, 1 : 1 + W] = x.astype(NP_BF16)
    # expert_weight [e, co, ci, kh, kw] -> [ci, e, kh*kw, co], bf16
    ew_t = np.ascontiguousarray(
        expert_weight.transpose(2, 0, 3, 4, 1)
        .reshape(CIN, E, KH * KW, COUT)
        .astype(NP_BF16)
    )
    rw_t = np.ascontiguousarray(routing_w.T)
    return [
        {
            "x": np.ascontiguousarray(xp[c * B_SH : (c + 1) * B_SH]),
            "routing_wt": rw_t,
            "routing_b": routing_b,
            "expert_weight_t": ew_t,
        }
        for c in range(N_CORES)
    ]


def kernel(x, routing_w, routing_b, expert_weight):
    nc = build_nc()
    in_maps = make_in_maps(x, routing_w, routing_b, expert_weight)
    res = run_bass_kernel_spmd(nc, in_maps, core_ids=list(range(N_CORES)))
    return np.concatenate(
        [res.results[c]["y"].astype(np.float32) for c in range(N_CORES)], axis=0
    )
